# revision 3
# baseline (speedup 1.0000x reference)
"""Trainium2 Bass kernel for nn_DCTLayer: per-8x8-block 2D DCT-like transform.

Math: reference computes, per 8x8 block X of the 256x256 image,
    out_block[y, v] = sum_x A[v, x] * X[x, y],   where A = D @ D
(D = 8x8 DCT basis). out_block = (A @ X)^T.

Kernel strategy (per core, pure data parallel over batch):
  - Load 128 consecutive image rows into SBUF naturally: partition = (G, x)
    [G = row-block, x = row-within-block], free = (J, y) [J = col-block,
    y = col-within-block].  Fully contiguous DMA.
  - One matmul per 128x128 tile with the DATA as the stationary operand and a
    constant 128x128 block-diagonal matrix BD (16 copies of A^T on the
    diagonal) as the moving operand:
        Z[(J,y), (G,v)] = sum_{G,x} X[(G,x),(J,y)] * A[v,x]
  - Store Z to DRAM with a 4D strided access pattern that places element
    (J,y,G,v) at output row 8G+y, col 8J+v.  This performs the within-block
    transpose during the store (32-byte contiguous runs).
"""

import sys

sys.path.insert(0, "/opt/trn_rl_repo")

from contextlib import ExitStack

import numpy as np

import concourse.bass as bass  # noqa: F401
import concourse.tile as tile
from concourse import bacc, mybir
from concourse.bass_utils import run_bass_kernel_spmd

P = 8
H = W = 256
B, C = 16, 64
NCORES = 8
BPC = B // NCORES  # batches per core
IMGS = BPC * C  # images (b,c planes) per core
ROWS = IMGS * H  # dram rows per core

TRACE = False
LAST_RESULTS = None

_nc_cache = None


def _dct_kernel(tc, o, x, bd):
    nc = tc.nc
    with ExitStack() as ctx:
        xpool = ctx.enter_context(tc.tile_pool(name="xin", bufs=6))
        zpool = ctx.enter_context(tc.tile_pool(name="zout", bufs=6))
        cpool = ctx.enter_context(tc.tile_pool(name="const", bufs=1))
        ppool = ctx.enter_context(tc.tile_pool(name="ps", bufs=8, space="PSUM"))

        bdt = cpool.tile([128, 128], mybir.dt.float32)
        nc.sync.dma_start(bdt[:], bd[:])

        for img in range(IMGS):
            # ---- load image (256x256) as [128, (r, c)] ----
            xt = xpool.tile([128, 2 * W], mybir.dt.float32)
            src = x[img * H : (img + 1) * H, :].rearrange("(r p) c -> p r c", p=128)
            dst = xt[:].rearrange("p (r c) -> p r c", c=W)
            nc.sync.dma_start(dst, src)

            # ---- 4 matmuls into one PSUM bank: quarter q = (r, h) ----
            ps = ppool.tile([128, 512], mybir.dt.float32)
            for r in range(2):
                for h in range(2):
                    q = r * 2 + h
                    nc.tensor.matmul(
                        ps[:, q * 128 : (q + 1) * 128],
                        xt[:, r * W + h * 128 : r * W + (h + 1) * 128],
                        bdt[:],
                        start=True,
                        stop=True,
                    )

            # ---- PSUM -> SBUF with (r,h) -> (h,r) quarter swap ----
            zt = zpool.tile([128, 512], mybir.dt.float32)
            zsrc = ps[:].rearrange("p (r h c) -> p r h c", r=2, h=2)
            zdst = zt[:].rearrange("p (h r c) -> p r h c", h=2, r=2)
            nc.vector.tensor_copy(zdst, zsrc)

            # ---- strided store performing the within-block transpose ----
            # zt half h: [p=(J,y), f=(G,v)] with G = 0..31 over both row-chunks
            # DMA APs are limited to 3 dims -> one DMA per y (rows 8G+y).
            for h in range(2):
                for y in range(P):
                    dstore = o[
                        img * H + y : (img + 1) * H : P, h * 128 : (h + 1) * 128
                    ].rearrange("G (J v) -> J G v", v=P)
                    nc.sync.dma_start(dstore, zt[y::P, h * 256 : (h + 1) * 256])


def _build_nc():
    nc = bacc.Bacc(
        "TRN2", target_bir_lowering=False, debug=False, num_devices=NCORES
    )
    x_ap = nc.dram_tensor("x", [ROWS, W], mybir.dt.float32, kind="ExternalInput").ap()
    bd_ap = nc.dram_tensor(
        "bd", [128, 128], mybir.dt.float32, kind="ExternalInput"
    ).ap()
    o_ap = nc.dram_tensor("o", [ROWS, W], mybir.dt.float32, kind="ExternalOutput").ap()
    with tile.TileContext(nc) as tc:
        _dct_kernel(tc, o_ap, x_ap, bd_ap)
    nc.compile()
    return nc


def _make_bd(dct_basis: np.ndarray) -> np.ndarray:
    a = dct_basis.astype(np.float64) @ dct_basis.astype(np.float64)
    at = a.T.astype(np.float32)  # block[x, v] = A[v, x]
    bd = np.zeros((128, 128), dtype=np.float32)
    for g in range(16):
        bd[g * P : (g + 1) * P, g * P : (g + 1) * P] = at
    return bd


def kernel(x: np.ndarray, dct_basis: np.ndarray) -> np.ndarray:
    global _nc_cache, LAST_RESULTS
    x = np.asarray(x, dtype=np.float32)
    dct_basis = np.asarray(dct_basis, dtype=np.float32)
    assert x.shape == (B, C, H, W)

    if _nc_cache is None:
        _nc_cache = _build_nc()
    nc = _nc_cache

    bd = _make_bd(dct_basis)
    in_maps = []
    for i in range(NCORES):
        xs = np.ascontiguousarray(x[i * BPC : (i + 1) * BPC]).reshape(ROWS, W)
        in_maps.append({"x": xs, "bd": bd})

    try:
        res = run_bass_kernel_spmd(
            nc, in_maps, core_ids=list(range(NCORES)), trace=TRACE
        )
    except ModuleNotFoundError:
        res = run_bass_kernel_spmd(
            nc, in_maps, core_ids=list(range(NCORES)), trace=False
        )
    LAST_RESULTS = res

    out = np.empty((B, C, H, W), dtype=np.float32)
    for i in range(NCORES):
        out[i * BPC : (i + 1) * BPC] = res.results[i]["o"].reshape(BPC, C, H, W)
    return out


# revision 5
# speedup vs baseline: 10399.5728x; 10399.5728x over previous
"""Trainium2 Bass kernel for nn_DCTLayer: per-8x8-block 2D DCT-like transform.

Math: reference computes, per 8x8 block X of the 256x256 image,
    out_block[y, v] = sum_x A[v, x] * X[x, y],   where A = D @ D
(D = 8x8 DCT basis). out_block = (A @ X)^T.

Kernel strategy (per core, pure data parallel over batch):
  - Load 128 consecutive image rows into SBUF naturally: partition = (G, x)
    [G = row-block, x = row-within-block], free = (J, y) [J = col-block,
    y = col-within-block].  Fully contiguous DMA.
  - One matmul per 128x128 tile with the DATA as the stationary operand and a
    constant 128x128 block-diagonal matrix BD (16 copies of A^T on the
    diagonal) as the moving operand:
        Z[(J,y), (G,v)] = sum_{G,x} X[(G,x),(J,y)] * A[v,x]
  - Store Z to DRAM with a 4D strided access pattern that places element
    (J,y,G,v) at output row 8G+y, col 8J+v.  This performs the within-block
    transpose during the store (32-byte contiguous runs).
"""

import sys

sys.path.insert(0, "/opt/trn_rl_repo")

from contextlib import ExitStack

import numpy as np

import concourse.bass as bass  # noqa: F401
import concourse.tile as tile
from concourse import bacc, mybir
from concourse.bass_utils import run_bass_kernel_spmd

P = 8
H = W = 256
B, C = 16, 64
NCORES = 8
BPC = B // NCORES  # batches per core
IMGS = BPC * C  # images (b,c planes) per core
ROWS = IMGS * H  # dram rows per core

TRACE = False
LAST_RESULTS = None

_nc_cache = None


def _ensure_ntff_hook():
    """The agent image's antenv lacks axon_hooks; synthesize it so
    run_bass_kernel_spmd(trace=True) can capture NTFF profiles."""
    import types

    if "antenv.axon_hooks" in sys.modules:
        return
    try:
        sys.path.insert(0, "/root/.axon_site/trn_agent_boot")
        from trn_boot import _ntff_profile_via_ctypes

        hook = _ntff_profile_via_ctypes("/opt/axon/libaxon_pjrt.so")
    except Exception:
        hook = None
    mod = types.ModuleType("antenv.axon_hooks")
    mod._hook = hook
    mod.get_axon_ntff_profile_hook = lambda: mod._hook
    mod.set_axon_ntff_profile_hook = lambda h: setattr(mod, "_hook", h)
    sys.modules["antenv.axon_hooks"] = mod


def _dct_kernel(tc, o, x, bd):
    nc = tc.nc
    with ExitStack() as ctx:
        xpool = ctx.enter_context(tc.tile_pool(name="xin", bufs=6))
        zpool = ctx.enter_context(tc.tile_pool(name="zout", bufs=6))
        cpool = ctx.enter_context(tc.tile_pool(name="const", bufs=1))
        ppool = ctx.enter_context(tc.tile_pool(name="ps", bufs=8, space="PSUM"))

        bdt = cpool.tile([128, 128], mybir.dt.float32)
        nc.sync.dma_start(bdt[:], bd[:])

        for img in range(IMGS):
            # ---- load image (256x256) as [128, (r, c)] ----
            xt = xpool.tile([128, 2 * W], mybir.dt.float32)
            src = x[img * H : (img + 1) * H, :].rearrange("(r p) c -> p r c", p=128)
            dst = xt[:].rearrange("p (r c) -> p r c", c=W)
            nc.sync.dma_start(dst, src)

            # ---- 4 matmuls into one PSUM bank: quarter q = (r, h) ----
            ps = ppool.tile([128, 512], mybir.dt.float32)
            for r in range(2):
                for h in range(2):
                    q = r * 2 + h
                    nc.tensor.matmul(
                        ps[:, q * 128 : (q + 1) * 128],
                        xt[:, r * W + h * 128 : r * W + (h + 1) * 128],
                        bdt[:],
                        start=True,
                        stop=True,
                    )

            # ---- PSUM -> SBUF with (r,h) -> (h,r) quarter swap ----
            zt = zpool.tile([128, 512], mybir.dt.float32)
            zsrc = ps[:].rearrange("p (r h c) -> p r h c", r=2, h=2)
            zdst = zt[:].rearrange("p (h r c) -> p r h c", h=2, r=2)
            nc.vector.tensor_copy(zdst, zsrc)

            # ---- strided store performing the within-block transpose ----
            # zt half h: [p=(J,y), f=(G,v)] with G = 0..31 over both row-chunks
            # DMA APs are limited to 3 dims -> one DMA per y (rows 8G+y).
            for h in range(2):
                for y in range(P):
                    dstore = o[
                        img * H + y : (img + 1) * H : P, h * 128 : (h + 1) * 128
                    ].rearrange("G (J v) -> J G v", v=P)
                    nc.sync.dma_start(dstore, zt[y::P, h * 256 : (h + 1) * 256])


def _build_nc():
    nc = bacc.Bacc(
        "TRN2", target_bir_lowering=False, debug=False, num_devices=NCORES
    )
    x_ap = nc.dram_tensor("x", [ROWS, W], mybir.dt.float32, kind="ExternalInput").ap()
    bd_ap = nc.dram_tensor(
        "bd", [128, 128], mybir.dt.float32, kind="ExternalInput"
    ).ap()
    o_ap = nc.dram_tensor("o", [ROWS, W], mybir.dt.float32, kind="ExternalOutput").ap()
    with tile.TileContext(nc) as tc:
        _dct_kernel(tc, o_ap, x_ap, bd_ap)
    nc.compile()
    return nc


def _make_bd(dct_basis: np.ndarray) -> np.ndarray:
    a = dct_basis.astype(np.float64) @ dct_basis.astype(np.float64)
    at = a.T.astype(np.float32)  # block[x, v] = A[v, x]
    bd = np.zeros((128, 128), dtype=np.float32)
    for g in range(16):
        bd[g * P : (g + 1) * P, g * P : (g + 1) * P] = at
    return bd


def kernel(x: np.ndarray, dct_basis: np.ndarray) -> np.ndarray:
    global _nc_cache, LAST_RESULTS
    x = np.asarray(x, dtype=np.float32)
    dct_basis = np.asarray(dct_basis, dtype=np.float32)
    assert x.shape == (B, C, H, W)

    if _nc_cache is None:
        _nc_cache = _build_nc()
    nc = _nc_cache

    bd = _make_bd(dct_basis)
    in_maps = []
    for i in range(NCORES):
        xs = np.ascontiguousarray(x[i * BPC : (i + 1) * BPC]).reshape(ROWS, W)
        in_maps.append({"x": xs, "bd": bd})

    if TRACE:
        _ensure_ntff_hook()
    try:
        res = run_bass_kernel_spmd(
            nc, in_maps, core_ids=list(range(NCORES)), trace=TRACE
        )
    except ModuleNotFoundError:
        res = run_bass_kernel_spmd(
            nc, in_maps, core_ids=list(range(NCORES)), trace=False
        )
    LAST_RESULTS = res

    out = np.empty((B, C, H, W), dtype=np.float32)
    for i in range(NCORES):
        out[i * BPC : (i + 1) * BPC] = res.results[i]["o"].reshape(BPC, C, H, W)
    return out


# revision 7
# speedup vs baseline: 14250.6773x; 1.3703x over previous
"""Trainium2 Bass kernel for nn_DCTLayer: per-8x8-block 2D DCT-like transform.

Math: reference computes, per 8x8 block X of the 256x256 image,
    out_block[y, v] = sum_x A[v, x] * X[x, y],   where A = D @ D
(D = 8x8 DCT basis). out_block = (A @ X)^T.

Kernel strategy (per core, pure data parallel over batch):
  - Load 128 consecutive image rows into SBUF naturally: partition = (G, x)
    [G = row-block, x = row-within-block], free = (J, y) [J = col-block,
    y = col-within-block].  Fully contiguous DMA.
  - One matmul per 128x128 tile with the DATA as the stationary operand and a
    constant 128x128 block-diagonal matrix BD (16 copies of A^T on the
    diagonal) as the moving operand:
        Z[(J,y), (G,v)] = sum_{G,x} X[(G,x),(J,y)] * A[v,x]
  - Store Z to DRAM with a 4D strided access pattern that places element
    (J,y,G,v) at output row 8G+y, col 8J+v.  This performs the within-block
    transpose during the store (32-byte contiguous runs).
"""

import sys

sys.path.insert(0, "/opt/trn_rl_repo")

from contextlib import ExitStack

import numpy as np

import concourse.bass as bass  # noqa: F401
import concourse.tile as tile
from concourse import bacc, mybir
from concourse.bass_utils import run_bass_kernel_spmd

P = 8
H = W = 256
B, C = 16, 64
NCORES = 8
BPC = B // NCORES  # batches per core
IMGS = BPC * C  # images (b,c planes) per core
ROWS = IMGS * H  # dram rows per core

TRACE = False
LAST_RESULTS = None

_nc_cache = None


def _ensure_ntff_hook():
    """The agent image's antenv lacks axon_hooks; synthesize it so
    run_bass_kernel_spmd(trace=True) can capture NTFF profiles."""
    import types

    if "antenv.axon_hooks" in sys.modules:
        return
    try:
        sys.path.insert(0, "/root/.axon_site/trn_agent_boot")
        from trn_boot import _ntff_profile_via_ctypes

        hook = _ntff_profile_via_ctypes("/opt/axon/libaxon_pjrt.so")
    except Exception:
        hook = None
    mod = types.ModuleType("antenv.axon_hooks")
    mod._hook = hook
    mod.get_axon_ntff_profile_hook = lambda: mod._hook
    mod.set_axon_ntff_profile_hook = lambda h: setattr(mod, "_hook", h)
    sys.modules["antenv.axon_hooks"] = mod


def _dct_kernel(tc, o, x, bd):
    nc = tc.nc
    with ExitStack() as ctx:
        xpool = ctx.enter_context(tc.tile_pool(name="xin", bufs=6))
        zpool = ctx.enter_context(tc.tile_pool(name="zout", bufs=6))
        cpool = ctx.enter_context(tc.tile_pool(name="const", bufs=1))
        ppool = ctx.enter_context(tc.tile_pool(name="ps", bufs=8, space="PSUM"))

        bdt = cpool.tile([128, 128], mybir.dt.float32)
        nc.sync.dma_start(bdt[:], bd[:])

        for img in range(IMGS):
            # ---- load image (256x256) as [128, (r, c)] ----
            xt = xpool.tile([128, 2 * W], mybir.dt.float32)
            src = x[img * H : (img + 1) * H, :].rearrange("(r p) c -> p r c", p=128)
            dst = xt[:].rearrange("p (r c) -> p r c", c=W)
            nc.gpsimd.dma_start(dst, src)

            # ---- 4 matmuls into one PSUM bank: quarter q = (r, h) ----
            ps = ppool.tile([128, 512], mybir.dt.float32)
            for r in range(2):
                for h in range(2):
                    q = r * 2 + h
                    nc.tensor.matmul(
                        ps[:, q * 128 : (q + 1) * 128],
                        xt[:, r * W + h * 128 : r * W + (h + 1) * 128],
                        bdt[:],
                        start=True,
                        stop=True,
                    )

            # ---- PSUM -> SBUF with (r,h) -> (h,r) quarter swap ----
            zt = zpool.tile([128, 512], mybir.dt.float32)
            zsrc = ps[:].rearrange("p (r h c) -> p r h c", r=2, h=2)
            zdst = zt[:].rearrange("p (h r c) -> p r h c", h=2, r=2)
            nc.vector.tensor_copy(zdst, zsrc)

            # ---- strided store performing the within-block transpose ----
            # zt half h: [p=(J,y), f=(G,v)] with G = 0..31 over both row-chunks
            # DMA APs are limited to 3 dims -> one DMA per y (rows 8G+y).
            # split descriptor generation across both HWDGE issuers (SP + ACT)
            for h in range(2):
                for y in range(P):
                    dstore = o[
                        img * H + y : (img + 1) * H : P, h * 128 : (h + 1) * 128
                    ].rearrange("G (J v) -> J G v", v=P)
                    eng = nc.sync if (y % 2 == 0) else nc.scalar
                    eng.dma_start(dstore, zt[y::P, h * 256 : (h + 1) * 256])


def _build_nc():
    nc = bacc.Bacc(
        "TRN2", target_bir_lowering=False, debug=False, num_devices=NCORES
    )
    x_ap = nc.dram_tensor("x", [ROWS, W], mybir.dt.float32, kind="ExternalInput").ap()
    bd_ap = nc.dram_tensor(
        "bd", [128, 128], mybir.dt.float32, kind="ExternalInput"
    ).ap()
    o_ap = nc.dram_tensor("o", [ROWS, W], mybir.dt.float32, kind="ExternalOutput").ap()
    with tile.TileContext(nc) as tc:
        _dct_kernel(tc, o_ap, x_ap, bd_ap)
    nc.compile()
    return nc


def _make_bd(dct_basis: np.ndarray) -> np.ndarray:
    a = dct_basis.astype(np.float64) @ dct_basis.astype(np.float64)
    at = a.T.astype(np.float32)  # block[x, v] = A[v, x]
    bd = np.zeros((128, 128), dtype=np.float32)
    for g in range(16):
        bd[g * P : (g + 1) * P, g * P : (g + 1) * P] = at
    return bd


def kernel(x: np.ndarray, dct_basis: np.ndarray) -> np.ndarray:
    global _nc_cache, LAST_RESULTS
    x = np.asarray(x, dtype=np.float32)
    dct_basis = np.asarray(dct_basis, dtype=np.float32)
    assert x.shape == (B, C, H, W)

    if _nc_cache is None:
        _nc_cache = _build_nc()
    nc = _nc_cache

    bd = _make_bd(dct_basis)
    in_maps = []
    for i in range(NCORES):
        xs = np.ascontiguousarray(x[i * BPC : (i + 1) * BPC]).reshape(ROWS, W)
        in_maps.append({"x": xs, "bd": bd})

    if TRACE:
        _ensure_ntff_hook()
    try:
        res = run_bass_kernel_spmd(
            nc, in_maps, core_ids=list(range(NCORES)), trace=TRACE
        )
    except ModuleNotFoundError:
        res = run_bass_kernel_spmd(
            nc, in_maps, core_ids=list(range(NCORES)), trace=False
        )
    LAST_RESULTS = res

    out = np.empty((B, C, H, W), dtype=np.float32)
    for i in range(NCORES):
        out[i * BPC : (i + 1) * BPC] = res.results[i]["o"].reshape(BPC, C, H, W)
    return out


# revision 9
# speedup vs baseline: 16915.7707x; 1.1870x over previous
"""Trainium2 Bass kernel for nn_DCTLayer: per-8x8-block 2D DCT-like transform.

Math: reference computes, per 8x8 block X of the 256x256 image,
    out_block[y, v] = sum_x A[v, x] * X[x, y],   where A = D @ D
(D = 8x8 DCT basis). out_block = (A @ X)^T.

Kernel strategy (per core, pure data parallel over batch):
  - Load 128 consecutive image rows into SBUF naturally: partition = (G, x)
    [G = row-block, x = row-within-block], free = (J, y) [J = col-block,
    y = col-within-block].  Fully contiguous DMA.
  - One matmul per 128x128 tile with the DATA as the stationary operand and a
    constant 128x128 block-diagonal matrix BD (16 copies of A^T on the
    diagonal) as the moving operand:
        Z[(J,y), (G,v)] = sum_{G,x} X[(G,x),(J,y)] * A[v,x]
  - Store Z to DRAM with a 4D strided access pattern that places element
    (J,y,G,v) at output row 8G+y, col 8J+v.  This performs the within-block
    transpose during the store (32-byte contiguous runs).
"""

import sys

sys.path.insert(0, "/opt/trn_rl_repo")

from contextlib import ExitStack

import numpy as np

import concourse.bass as bass  # noqa: F401
import concourse.tile as tile
from concourse import bacc, mybir
from concourse.bass_utils import run_bass_kernel_spmd

P = 8
H = W = 256
B, C = 16, 64
NCORES = 8
BPC = B // NCORES  # batches per core
IMGS = BPC * C  # images (b,c planes) per core
ROWS = IMGS * H  # dram rows per core

TRACE = False
LAST_RESULTS = None

_nc_cache = None


def _ensure_ntff_hook():
    """The agent image's antenv lacks axon_hooks; synthesize it so
    run_bass_kernel_spmd(trace=True) can capture NTFF profiles."""
    import types

    if "antenv.axon_hooks" in sys.modules:
        return
    try:
        sys.path.insert(0, "/root/.axon_site/trn_agent_boot")
        from trn_boot import _ntff_profile_via_ctypes

        hook = _ntff_profile_via_ctypes("/opt/axon/libaxon_pjrt.so")
    except Exception:
        hook = None
    mod = types.ModuleType("antenv.axon_hooks")
    mod._hook = hook
    mod.get_axon_ntff_profile_hook = lambda: mod._hook
    mod.set_axon_ntff_profile_hook = lambda h: setattr(mod, "_hook", h)
    sys.modules["antenv.axon_hooks"] = mod


def _dct_kernel(tc, o, x, bd):
    nc = tc.nc
    with ExitStack() as ctx:
        xpool = ctx.enter_context(tc.tile_pool(name="xin", bufs=10))
        zpool = ctx.enter_context(tc.tile_pool(name="zout", bufs=10))
        cpool = ctx.enter_context(tc.tile_pool(name="const", bufs=1))
        ppool = ctx.enter_context(tc.tile_pool(name="ps", bufs=8, space="PSUM"))

        bdt = cpool.tile([128, 128], mybir.dt.float32)
        nc.sync.dma_start(bdt[:], bd[:])

        for img in range(IMGS):
            # ---- load image (256x256) as [128, (r, c)] ----
            xt = xpool.tile([128, 2 * W], mybir.dt.float32)
            src = x[img * H : (img + 1) * H, :].rearrange("(r p) c -> p r c", p=128)
            dst = xt[:].rearrange("p (r c) -> p r c", c=W)
            nc.gpsimd.dma_start(dst, src)

            # ---- 4 matmuls into one PSUM bank: quarter q = (r, h) ----
            ps = ppool.tile([128, 512], mybir.dt.float32)
            for r in range(2):
                for h in range(2):
                    q = r * 2 + h
                    nc.tensor.matmul(
                        ps[:, q * 128 : (q + 1) * 128],
                        xt[:, r * W + h * 128 : r * W + (h + 1) * 128],
                        bdt[:],
                        start=True,
                        stop=True,
                    )

            # ---- PSUM -> SBUF with (r,h) -> (h,r) quarter swap ----
            zt = zpool.tile([128, 512], mybir.dt.float32)
            zsrc = ps[:].rearrange("p (r h c) -> p r h c", r=2, h=2)
            zdst = zt[:].rearrange("p (h r c) -> p r h c", h=2, r=2)
            nc.vector.tensor_copy(zdst, zsrc)

            # ---- strided store performing the within-block transpose ----
            # zt half h: [p=(J,y), f=(G,v)] with G = 0..31 over both row-chunks
            # DMA APs are limited to 3 dims -> one DMA per y (rows 8G+y).
            # split descriptor generation across all three DGE issuers.
            # HWDGE (SP/ACT) ~1.25ns/desc, SWDGE (Pool) ~3ns/desc -> 7/7/2.
            for h in range(2):
                for y in range(P):
                    i = h * P + y
                    if i < 7:
                        eng = nc.sync
                    elif i < 14:
                        eng = nc.scalar
                    else:
                        eng = nc.gpsimd
                    dstore = o[
                        img * H + y : (img + 1) * H : P, h * 128 : (h + 1) * 128
                    ].rearrange("G (J v) -> J G v", v=P)
                    eng.dma_start(dstore, zt[y::P, h * 256 : (h + 1) * 256])


def _build_nc():
    nc = bacc.Bacc(
        "TRN2", target_bir_lowering=False, debug=False, num_devices=NCORES
    )
    x_ap = nc.dram_tensor("x", [ROWS, W], mybir.dt.float32, kind="ExternalInput").ap()
    bd_ap = nc.dram_tensor(
        "bd", [128, 128], mybir.dt.float32, kind="ExternalInput"
    ).ap()
    o_ap = nc.dram_tensor("o", [ROWS, W], mybir.dt.float32, kind="ExternalOutput").ap()
    with tile.TileContext(nc) as tc:
        _dct_kernel(tc, o_ap, x_ap, bd_ap)
    nc.compile()
    return nc


def _make_bd(dct_basis: np.ndarray) -> np.ndarray:
    a = dct_basis.astype(np.float64) @ dct_basis.astype(np.float64)
    at = a.T.astype(np.float32)  # block[x, v] = A[v, x]
    bd = np.zeros((128, 128), dtype=np.float32)
    for g in range(16):
        bd[g * P : (g + 1) * P, g * P : (g + 1) * P] = at
    return bd


def kernel(x: np.ndarray, dct_basis: np.ndarray) -> np.ndarray:
    global _nc_cache, LAST_RESULTS
    x = np.asarray(x, dtype=np.float32)
    dct_basis = np.asarray(dct_basis, dtype=np.float32)
    assert x.shape == (B, C, H, W)

    if _nc_cache is None:
        _nc_cache = _build_nc()
    nc = _nc_cache

    bd = _make_bd(dct_basis)
    in_maps = []
    for i in range(NCORES):
        xs = np.ascontiguousarray(x[i * BPC : (i + 1) * BPC]).reshape(ROWS, W)
        in_maps.append({"x": xs, "bd": bd})

    if TRACE:
        _ensure_ntff_hook()
    try:
        res = run_bass_kernel_spmd(
            nc, in_maps, core_ids=list(range(NCORES)), trace=TRACE
        )
    except ModuleNotFoundError:
        res = run_bass_kernel_spmd(
            nc, in_maps, core_ids=list(range(NCORES)), trace=False
        )
    LAST_RESULTS = res

    out = np.empty((B, C, H, W), dtype=np.float32)
    for i in range(NCORES):
        out[i * BPC : (i + 1) * BPC] = res.results[i]["o"].reshape(BPC, C, H, W)
    return out
